# revision 15
# baseline (speedup 1.0000x reference)
"""CryptEAGLE GNN message-passing layer on 8 Trainium2 NeuronCores.

Sharding: nodes split into 8 contiguous dst-ranges of 6250; edges live on the
core owning their dst, so both segment-sums (row_sum, agg) are core-local.
Only a [128,2] GraphNorm-stats AllReduce crosses cores.

Per core, everything is streamed (no device-side gather): the host pre-lays
per-edge source features x[src], edge_attr, and the two window one-hots in
channel-major tile order.  Per 128-edge tile the device computes
  kvpe = x_src @ [WK.T|WV.T] + eattr @ [WE.T|WE.T]   (PSUM accumulation)
  qe   = onehotT.T @ q_window                         (q broadcast to edges)
  score= relu(sum_h qe*kpe), msg = score*(v+e)        (DVE, reading PSUM)
  agg  += onehot.T @ [msg|score]                      (segment-sum matmul)
Aggregation matmuls are skewed one macro behind the score/msg DVE work so
the tensor engine never waits on the vector engine.  Output phase runs
transposed (channels on partitions) so GraphNorm's per-channel affine is one
per-partition tensor_scalar; host transposes back.
"""

import numpy as np
import ml_dtypes

import concourse.bass as bass
import concourse.mybir as mybir
import concourse.tile as tile
import concourse.bacc as bacc
from concourse.bass_utils import run_bass_kernel_spmd

F32 = mybir.dt.float32
BF16 = mybir.dt.bfloat16
BF = ml_dtypes.bfloat16
AF = mybir.ActivationFunctionType
OP = mybir.AluOpType

N = 50000
E = 600000
DIM = 128
H = 8
HD = 16
NC = 8
NPC = N // NC
WIN = 128
NW = (NPC + WIN - 1) // WIN
NPAD = NW * WIN
MACRO = 2
CHUNK_TILES = 40  # max tiles per DMA chunk (SBUF budget)


def _plan(edge_index):
    src = np.asarray(edge_index[0], dtype=np.int64)
    dst = np.asarray(edge_index[1], dtype=np.int64)

    per = [[None] * NW for _ in range(NC)]
    core_of = dst // NPC
    for c in range(NC):
        m = core_of == c
        s_c, d_c = src[m], dst[m]
        eidx_c = np.nonzero(m)[0]
        rel = d_c - c * NPC
        w_c = rel // WIN
        order = np.argsort(w_c, kind="stable")
        s_c, rel, w_c, eidx_c = s_c[order], rel[order], w_c[order], eidx_c[order]
        bounds = np.searchsorted(w_c, np.arange(NW + 1))
        for w in range(NW):
            a, b = bounds[w], bounds[w + 1]
            per[c][w] = (s_c[a:b], rel[a:b] - w * WIN, eidx_c[a:b])

    nt = np.zeros(NW, np.int64)
    for w in range(NW):
        nt[w] = max(max(-(-len(per[c][w][0]) // 128) for c in range(NC)), 1)

    # chunk windows so each chunk is <= CHUNK_TILES tiles
    chunks = []  # (first_window, n_windows, tile_offset, n_tiles)
    w0, t0, toff = 0, 0, 0
    for w in range(NW):
        t = int(nt[w])
        if t0 and t0 + t > CHUNK_TILES:
            chunks.append((w0, w - w0, toff, t0))
            toff += t0
            w0, t0 = w, t
        else:
            t0 += t
    chunks.append((w0, NW - w0, toff, t0))

    starts = np.concatenate([[0], np.cumsum(nt)])
    return dict(per=per, nt=nt, starts=starts, chunks=chunks,
                total_tiles=int(nt.sum()))


def _per_core_arrays(plan, c, x_bf, ea_bf):
    nt, starts = plan["nt"], plan["starts"]
    T = plan["total_tiles"]
    xsrcT = np.zeros((128, T * 128), BF)
    eattrT = np.zeros((128, T * 128), BF)
    ohP = np.zeros((128, T * 128), BF)
    ohT = np.zeros((128, T * 128), BF)

    for w in range(NW):
        s, rel, eix = plan["per"][c][w]
        k = len(s)
        col0 = int(starts[w]) * 128
        xsrcT[:, col0:col0 + k] = x_bf[s].T
        eattrT[:, col0:col0 + k] = ea_bf[eix].T
        j = np.arange(k)
        ohP[j % 128, (int(starts[w]) + j // 128) * 128 + rel] = 1.0
        ohT[rel, col0 + j] = 1.0

    return dict(xsrcT=xsrcT, eattrT=eattrT, ohP=ohP, ohT=ohT)


def _build(plan):
    nt = plan["nt"]
    T = plan["total_tiles"]

    nc = bacc.Bacc("TRN2", target_bir_lowering=False, debug=False,
                   num_devices=NC)

    def din(name, shape, dt):
        return nc.declare_dram_parameter(name, list(shape), dt, isOutput=False)

    xTn = din("xTn", (128, NPAD), BF16)
    xsrcT = din("xsrcT", (128, T * 128), BF16)
    eattrT = din("eattrT", (128, T * 128), BF16)
    ohP_d = din("ohP", (128, T * 128), BF16)
    ohT_d = din("ohT", (128, T * 128), BF16)
    wkvT = din("wkvT", (128, 256), BF16)
    we2T = din("we2T", (128, 256), BF16)
    wqT = din("wqT", (128, 128), BF16)
    woT = din("woT", (128, 128), BF16)
    wpT = din("wpT", (128, 128), BF16)
    proto = din("proto", (128, 1), BF16)
    bo_c = din("bo", (128, 1), F32)
    gnw_c = din("gnw", (128, 1), F32)
    gnb_c = din("gnb", (128, 1), F32)
    gms_c = din("gms", (128, 1), F32)
    ones_row = din("ones_row", (1, 128), BF16)
    ident = din("ident", (128, 128), BF16)

    out_d = nc.declare_dram_parameter("out", [128, NPC], F32, isOutput=True)

    st_in = nc.dram_tensor("st_in", [128, 2], F32)
    st_out = nc.dram_tensor("st_out", [128, 2], F32, addr_space="Shared")

    with tile.TileContext(nc) as tc:
        with (
            tc.tile_pool(name="const", bufs=1) as cst,
            tc.tile_pool(name="persist", bufs=1) as psst,
        ):
            def const(name, dram, shape, dt):
                t = cst.tile(list(shape), dt, tag=name)
                nc.sync.dma_start(out=t[:], in_=dram[:])
                return t

            wkvT_s = const("wkvT", wkvT, (128, 256), BF16)
            we2T_s = const("we2T", we2T, (128, 256), BF16)
            wqT_s = const("wqT", wqT, (128, 128), BF16)
            woT_s = const("woT", woT, (128, 128), BF16)
            wpT_s = const("wpT", wpT, (128, 128), BF16)
            proto_s = const("proto", proto, (128, 1), BF16)
            ones_s = const("ones", ones_row, (1, 128), BF16)
            ident_s = const("ident", ident, (128, 128), BF16)
            bo_s = const("bo", bo_c, (128, 1), F32)
            gnw_s = const("gnw", gnw_c, (128, 1), F32)
            gnb_s = const("gnb", gnb_c, (128, 1), F32)
            gms_s = const("gms", gms_c, (128, 1), F32)

            outT = psst.tile([128, NPAD], F32, tag="outT")
            q_sb = psst.tile([128, NW, 128], BF16, tag="qsb")
            xr_sb = psst.tile([128, NPAD], BF16, tag="xrsb")
            nc.sync.dma_start(out=xr_sb[:], in_=xTn[:])

            # ---- q phase: q = x@WQ.T + 0.1*relu(q.p)*p, scaled 0.25 ----
            with (
                tc.tile_pool(name="qb", bufs=3) as qb,
                tc.tile_pool(name="qps", bufs=2, space="PSUM") as qps,
            ):
                p_ps = qps.tile([1, 128], F32, space="PSUM", tag="pps")
                nc.tensor.matmul(out=p_ps[:], lhsT=proto_s[:], rhs=wpT_s[:],
                                 start=True, stop=True)
                p_row = cst.tile([1, 128], BF16, tag="prow")
                nc.vector.tensor_copy(out=p_row[:], in_=p_ps[:])
                pbc_ps = qps.tile([128, 128], F32, space="PSUM", tag="pbcps")
                nc.tensor.matmul(out=pbc_ps[:], lhsT=ones_s[:], rhs=p_row[:],
                                 start=True, stop=True)
                pbc = cst.tile([128, 128], BF16, tag="pbc")
                nc.vector.tensor_copy(out=pbc[:], in_=pbc_ps[:])
                for t in range(NW):
                    q0 = qps.tile([128, 128], F32, space="PSUM")
                    nc.tensor.matmul(out=q0[:],
                                     lhsT=xr_sb[:, t * 128:(t + 1) * 128],
                                     rhs=wqT_s[:], start=True, stop=True)
                    pp = qb.tile([128, 128], F32, tag="pp")
                    nc.vector.tensor_tensor(out=pp[:], in0=q0[:], in1=pbc[:],
                                            op=OP.mult)
                    al = qb.tile([128, 8], F32, tag="al")
                    nc.vector.tensor_reduce(
                        out=al[:],
                        in_=pp[:].rearrange("p (h d) -> p h d", h=8),
                        axis=mybir.AxisListType.X, op=OP.add)
                    als = qb.tile([128, 8], F32, tag="als")
                    nc.scalar.activation(out=als[:], in_=al[:], func=AF.Relu,
                                         scale=0.1)
                    t2 = qb.tile([128, 128], F32, tag="t2")
                    nc.vector.tensor_tensor(
                        out=t2[:].rearrange("p (h d) -> p h d", h=8),
                        in0=pbc[:].rearrange("p (h d) -> p h d", h=8),
                        in1=als[:].rearrange("p (h o) -> p h o", o=1)
                        .to_broadcast([128, 8, 16]),
                        op=OP.mult)
                    q3 = qb.tile([128, 128], F32, tag="q3")
                    nc.vector.tensor_tensor(out=q3[:], in0=q0[:], in1=t2[:],
                                            op=OP.add)
                    nc.vector.tensor_scalar(out=q_sb[:, t, :], in0=q3[:],
                                            scalar1=0.25, scalar2=None,
                                            op0=OP.mult)

            # ---- edge phase -----------------------------------------
            with (
                tc.tile_pool(name="xs", bufs=2) as xsp,
                tc.tile_pool(name="ea", bufs=2) as eap,
                tc.tile_pool(name="oh", bufs=2) as ohp,
                tc.tile_pool(name="ohT", bufs=2) as ohTp,
                tc.tile_pool(name="wk", bufs=3) as wk,
                tc.tile_pool(name="wcl", bufs=2) as wcl,
                tc.tile_pool(name="kvps", bufs=3, space="PSUM") as kvps,
                tc.tile_pool(name="qeps", bufs=3, space="PSUM") as qeps,
                tc.tile_pool(name="aggp", bufs=2, space="PSUM") as aggp,
            ):
                pending_agg = []  # deferred agg matmuls (skewed one macro)
                pending_close = []  # deferred window-close ops

                def flush_agg():
                    for args in pending_agg:
                        aggbank_, oh_, ti_, wmsg_, t_, first_, last_ = args
                        nc.tensor.matmul(
                            out=aggbank_[:, 0:136], lhsT=oh_[:, ti_, :],
                            rhs=wmsg_[:, t_, :], start=first_, stop=last_)
                    pending_agg.clear()

                def flush_close():
                    for aggbank_, w_ in pending_close:
                        close_window(aggbank_, w_)
                    pending_close.clear()

                def close_window(aggbank, w):
                    rs_e = wcl.tile([128, 8], F32, tag="rse")
                    nc.vector.tensor_scalar(out=rs_e[:],
                                            in0=aggbank[:, 128:136],
                                            scalar1=1e-6, scalar2=None,
                                            op0=OP.add)
                    rinv = wcl.tile([128, 8], F32, tag="rinv")
                    nc.vector.reciprocal(out=rinv[:], in_=rs_e[:])
                    aggn = wcl.tile([128, 128], BF16, tag="aggn")
                    nc.vector.tensor_tensor(
                        out=aggn[:].rearrange("p (h d) -> p h d", h=8),
                        in0=aggbank[:, 0:128].rearrange("p (h d) -> p h d",
                                                        h=8),
                        in1=rinv[:].rearrange("p (h o) -> p h o", o=1)
                        .to_broadcast([128, 8, 16]),
                        op=OP.mult)
                    at_ps = aggbank[:, 192:256].bitcast(BF16)
                    nc.tensor.transpose(out=at_ps, in_=aggn[:],
                                        identity=ident_s[:])
                    at_sb = wcl.tile([128, 128], BF16, tag="atsb")
                    nc.scalar.activation(out=at_sb[:], in_=at_ps,
                                         func=AF.Copy)
                    ow_ps = aggbank[:, 256:384]
                    nc.tensor.matmul(out=ow_ps, lhsT=woT_s[:],
                                     rhs=at_sb[:], start=True, stop=True)
                    ow1 = wcl.tile([128, 128], F32, tag="ow1")
                    nc.scalar.activation(out=ow1[:], in_=ow_ps[:],
                                         func=AF.Identity, bias=bo_s[:, 0:1])
                    nc.vector.tensor_tensor(
                        out=outT[:, w * 128:(w + 1) * 128],
                        in0=ow1[:],
                        in1=xr_sb[:, w * 128:(w + 1) * 128], op=OP.add)

                for (w0, nwin, toff, ctiles) in plan["chunks"]:
                    c0 = toff * 128
                    cn = ctiles * 128
                    xs = xsp.tile([128, CHUNK_TILES * 128], BF16, tag="xs")
                    nc.sync.dma_start(out=xs[:, :cn],
                                      in_=xsrcT[:, c0:c0 + cn])
                    ea = eap.tile([128, CHUNK_TILES * 128], BF16, tag="ea")
                    nc.sync.dma_start(out=ea[:, :cn],
                                      in_=eattrT[:, c0:c0 + cn])
                    oh = ohp.tile([128, CHUNK_TILES, 128], BF16, tag="oh")
                    nc.sync.dma_start(
                        out=oh[:, :ctiles, :].rearrange("p t n -> p (t n)"),
                        in_=ohP_d[:, c0:c0 + cn])
                    ohT = ohTp.tile([128, CHUNK_TILES, 128], BF16, tag="ohT")
                    nc.sync.dma_start(
                        out=ohT[:, :ctiles, :].rearrange("p t n -> p (t n)"),
                        in_=ohT_d[:, c0:c0 + cn])

                    ct = 0  # tile index within chunk
                    for w in range(w0, w0 + nwin):
                        wt = int(nt[w])
                        aggbank = aggp.tile([128, 512], F32, space="PSUM")
                        wtile = 0
                        while wtile < wt:
                            mt = min(MACRO, wt - wtile)
                            kv_ps = kvps.tile([128, MACRO, 256], F32,
                                              space="PSUM")
                            for t in range(mt):
                                col = (ct + t) * 128
                                nc.tensor.matmul(
                                    out=kv_ps[:, t, :],
                                    lhsT=xs[:, col:col + 128],
                                    rhs=wkvT_s[:], start=True, stop=False)
                                nc.tensor.matmul(
                                    out=kv_ps[:, t, :],
                                    lhsT=ea[:, col:col + 128],
                                    rhs=we2T_s[:], start=False, stop=True)
                            qe_ps = qeps.tile([128, MACRO * 128], F32,
                                              space="PSUM")
                            for t in range(mt):
                                nc.tensor.matmul(
                                    out=qe_ps[:, t * 128:(t + 1) * 128],
                                    lhsT=ohT[:, ct + t, :],
                                    rhs=q_sb[:, w, :], start=True, stop=True)
                            # previous macro's aggregation (skewed)
                            flush_agg()
                            flush_close()

                            qe_sb = wk.tile([128, MACRO * 128], BF16,
                                            tag="qesb")
                            nc.scalar.activation(out=qe_sb[:, :mt * 128],
                                                 in_=qe_ps[:, :mt * 128],
                                                 func=AF.Copy)
                            prod = wk.tile([128, MACRO, 128], BF16,
                                           tag="prod")
                            nc.vector.tensor_tensor(
                                out=prod[:, :mt, :],
                                in0=qe_sb[:, :mt * 128]
                                .rearrange("p (t r) -> p t r", t=mt),
                                in1=kv_ps[:, :mt, 0:128],
                                op=OP.mult)
                            score = wk.tile([128, MACRO * 8], F32,
                                            tag="score")
                            nc.vector.tensor_reduce(
                                out=score[:, :mt * 8],
                                in_=prod[:, :mt, :]
                                .rearrange("p t (h d) -> p (t h) d", h=8),
                                axis=mybir.AxisListType.X, op=OP.add)
                            vpe_sb = wk.tile([128, MACRO, 128], BF16,
                                             tag="vpesb")
                            nc.vector.tensor_copy(out=vpe_sb[:, :mt, :],
                                                  in_=kv_ps[:, :mt, 128:256])
                            wmsg = wk.tile([128, MACRO, 136], BF16,
                                           tag="wmsg")
                            nc.scalar.activation(
                                out=wmsg[:, :mt, 128:136],
                                in_=score[:, :mt * 8]
                                .rearrange("p (t h) -> p t h", h=8),
                                func=AF.Relu)
                            nc.gpsimd.tensor_tensor(
                                out=wmsg[:, :mt, 0:128]
                                .rearrange("p t (h d) -> p t h d", h=8),
                                in0=vpe_sb[:, :mt, :]
                                .rearrange("p t (h d) -> p t h d", h=8),
                                in1=wmsg[:, :mt, 128:136]
                                .rearrange("p t (h o) -> p t h o", h=8, o=1)
                                .to_broadcast([128, mt, 8, 16]),
                                op=OP.mult)
                            for t in range(mt):
                                last = (wtile + t == wt - 1)
                                pending_agg.append(
                                    (aggbank, oh, ct + t, wmsg, t,
                                     wtile + t == 0, last))
                            wtile += mt
                            ct += mt
                        pending_close.append((aggbank, w))
                flush_agg()
                flush_close()

                # ---- GraphNorm -------------------------------------
                s1 = wcl.tile([128, 1], F32, tag="s1")
                nc.vector.tensor_reduce(out=s1[:], in_=outT[:, :NPC],
                                        axis=mybir.AxisListType.X, op=OP.add)
                s2 = wcl.tile([128, 1], F32, tag="s2")
                s2p = wcl.tile([128, 1], F32, tag="s2p")
                nc.vector.memset(s2[:], 0.0)
                for ch in range(0, NPC, 512):
                    cw = min(512, NPC - ch)
                    sq = wk.tile([128, 512], F32, tag="sq")
                    nc.scalar.activation(out=sq[:, :cw],
                                         in_=outT[:, ch:ch + cw],
                                         func=AF.Square)
                    nc.vector.tensor_reduce(out=s2p[:], in_=sq[:, :cw],
                                            axis=mybir.AxisListType.X,
                                            op=OP.add)
                    nc.vector.tensor_tensor(out=s2[:], in0=s2[:], in1=s2p[:],
                                            op=OP.add)
                st_sb = wcl.tile([128, 2], F32, tag="stsb")
                nc.vector.tensor_copy(out=st_sb[:, 0:1], in_=s1[:])
                nc.vector.tensor_copy(out=st_sb[:, 1:2], in_=s2[:])
                nc.sync.dma_start(out=st_in[:], in_=st_sb[:])
                nc.gpsimd.collective_compute(
                    "AllReduce", OP.add, replica_groups=[list(range(NC))],
                    ins=[st_in[:]], outs=[st_out[:]])
                stg = wcl.tile([128, 2], F32, tag="stg")
                nc.sync.dma_start(out=stg[:], in_=st_out[:])

                mean = wcl.tile([128, 1], F32, tag="mean")
                nc.vector.tensor_scalar(out=mean[:], in0=stg[:, 0:1],
                                        scalar1=1.0 / N, scalar2=None,
                                        op0=OP.mult)
                m2 = wcl.tile([128, 1], F32, tag="m2")
                nc.vector.tensor_scalar(out=m2[:], in0=stg[:, 1:2],
                                        scalar1=1.0 / N, scalar2=None,
                                        op0=OP.mult)
                gm = wcl.tile([128, 1], F32, tag="gm")
                nc.vector.tensor_tensor(out=gm[:], in0=gms_s[:], in1=mean[:],
                                        op=OP.mult)
                var = wcl.tile([128, 1], F32, tag="var")
                nc.vector.tensor_tensor(out=var[:], in0=gm[:], in1=gm[:],
                                        op=OP.mult)
                tmp = wcl.tile([128, 1], F32, tag="tmp")
                nc.vector.tensor_tensor(out=tmp[:], in0=gm[:], in1=mean[:],
                                        op=OP.mult)
                nc.vector.tensor_scalar(out=tmp[:], in0=tmp[:], scalar1=-2.0,
                                        scalar2=None, op0=OP.mult)
                nc.vector.tensor_tensor(out=var[:], in0=var[:], in1=tmp[:],
                                        op=OP.add)
                nc.vector.tensor_tensor(out=var[:], in0=var[:], in1=m2[:],
                                        op=OP.add)
                nc.vector.tensor_scalar(out=var[:], in0=var[:], scalar1=1e-5,
                                        scalar2=None, op0=OP.add)
                std = wcl.tile([128, 1], F32, tag="std")
                nc.scalar.sqrt(out=std[:], in_=var[:])
                rstd = wcl.tile([128, 1], F32, tag="rstd")
                nc.vector.reciprocal(out=rstd[:], in_=std[:])
                acol = wcl.tile([128, 1], F32, tag="acol")
                nc.vector.tensor_tensor(out=acol[:], in0=gnw_s[:],
                                        in1=rstd[:], op=OP.mult)
                bcol = wcl.tile([128, 1], F32, tag="bcol")
                nc.vector.tensor_tensor(out=bcol[:], in0=acol[:], in1=gm[:],
                                        op=OP.mult)
                nc.vector.tensor_scalar(out=bcol[:], in0=bcol[:],
                                        scalar1=-1.0, scalar2=None,
                                        op0=OP.mult)
                nc.vector.tensor_tensor(out=bcol[:], in0=bcol[:],
                                        in1=gnb_s[:], op=OP.add)

                fin = psst.tile([128, NPC], F32, tag="fin")
                nc.vector.tensor_scalar(out=fin[:], in0=outT[:, :NPC],
                                        scalar1=acol[:, 0:1],
                                        scalar2=bcol[:, 0:1],
                                        op0=OP.mult, op1=OP.add)
                nc.vector.tensor_scalar(out=fin[:], in0=fin[:], scalar1=0.0,
                                        scalar2=None, op0=OP.max)
                nc.sync.dma_start(out=out_d[:], in_=fin[:])

    nc.compile()
    return nc


def _in_maps(plan, x, edge_attr, prototype, WQ, WK, WV, WE, Wp, Wo, bo,
             gn_weight, gn_bias, gn_mean_scale):
    x_bf = np.asarray(x, np.float32).astype(BF)
    ea_bf = np.asarray(edge_attr, np.float32).astype(BF)
    wkvT = np.concatenate([np.asarray(WK, np.float32).T,
                           np.asarray(WV, np.float32).T], axis=1).astype(BF)
    weT = np.asarray(WE, np.float32).T.astype(BF)
    we2T = np.concatenate([weT, weT], axis=1)
    consts = dict(
        wkvT=np.ascontiguousarray(wkvT),
        we2T=np.ascontiguousarray(we2T),
        wqT=np.ascontiguousarray(np.asarray(WQ, np.float32).T).astype(BF),
        woT=np.ascontiguousarray(np.asarray(Wo, np.float32).T).astype(BF),
        wpT=np.ascontiguousarray(np.asarray(Wp, np.float32).T).astype(BF),
        proto=np.asarray(prototype, np.float32).reshape(128, 1).astype(BF),
        bo=np.asarray(bo, np.float32).reshape(128, 1),
        gnw=np.asarray(gn_weight, np.float32).reshape(128, 1),
        gnb=np.asarray(gn_bias, np.float32).reshape(128, 1),
        gms=np.asarray(gn_mean_scale, np.float32).reshape(128, 1),
        ones_row=np.ones((1, 128), BF),
        ident=np.eye(128, dtype=BF),
    )
    maps = []
    for c in range(NC):
        arrs = _per_core_arrays(plan, c, x_bf, ea_bf)
        pad = np.zeros((NPAD, 128), BF)
        pad[:NPC] = x_bf[c * NPC:(c + 1) * NPC]
        m = dict(consts)
        m["xTn"] = np.ascontiguousarray(pad.T)
        m["xsrcT"] = arrs["xsrcT"]
        m["eattrT"] = arrs["eattrT"]
        m["ohP"] = arrs["ohP"]
        m["ohT"] = arrs["ohT"]
        maps.append(m)
    return maps


def kernel(x, edge_attr, prototype, WQ, WK, WV, WE, Wp, Wo, bo,
           gn_weight, gn_bias, gn_mean_scale, edge_index):
    x = np.asarray(x, np.float32)
    edge_attr = np.asarray(edge_attr, np.float32)
    plan = _plan(np.asarray(edge_index))
    nc = _build(plan)
    maps = _in_maps(plan, x, edge_attr, prototype, WQ, WK, WV, WE, Wp, Wo,
                    bo, gn_weight, gn_bias, gn_mean_scale)
    res = run_bass_kernel_spmd(nc, maps, list(range(NC)), trace=False)
    out = np.empty((N, DIM), np.float32)
    for c in range(NC):
        out[c * NPC:(c + 1) * NPC] = res.results[c]["out"].T
    return out


# revision 18
# speedup vs baseline: 1.1788x; 1.1788x over previous
"""CryptEAGLE GNN message-passing layer on 8 Trainium2 NeuronCores.

Sharding: nodes split into 8 contiguous dst-ranges of 6250; edges live on the
core owning their dst, so both segment-sums (row_sum, agg) are core-local.
Only a [128,2] GraphNorm-stats AllReduce crosses cores.

Per core, everything is streamed (no device-side gather): the host pre-lays
per-edge source features x[src], edge_attr, and the two window one-hots in
channel-major tile order.  Per 128-edge tile the device computes
  kvpe = x_src @ [WK.T|WV.T] + eattr @ [WE.T|WE.T]   (PSUM accumulation)
  qe   = onehotT.T @ q_window                         (q broadcast to edges)
  score= relu(sum_h qe*kpe), msg = score*(v+e)        (DVE, reading PSUM)
  agg  += onehot.T @ [msg|score]                      (segment-sum matmul)
Aggregation matmuls are skewed one macro behind the score/msg DVE work so
the tensor engine never waits on the vector engine.  Output phase runs
transposed (channels on partitions) so GraphNorm's per-channel affine is one
per-partition tensor_scalar; host transposes back.
"""

import numpy as np
import ml_dtypes

import concourse.bass as bass
import concourse.mybir as mybir
import concourse.tile as tile
import concourse.bacc as bacc
from concourse.bass_utils import run_bass_kernel_spmd

F32 = mybir.dt.float32
BF16 = mybir.dt.bfloat16
BF = ml_dtypes.bfloat16
AF = mybir.ActivationFunctionType
OP = mybir.AluOpType

N = 50000
E = 600000
DIM = 128
H = 8
HD = 16
NC = 8
NPC = N // NC
WIN = 128
NW = (NPC + WIN - 1) // WIN
NPAD = NW * WIN
MACRO = 2
CHUNK_TILES = 40  # max tiles per DMA chunk (SBUF budget)


def _plan(edge_index):
    src = np.asarray(edge_index[0], dtype=np.int64)
    dst = np.asarray(edge_index[1], dtype=np.int64)

    per = [[None] * NW for _ in range(NC)]
    core_of = dst // NPC
    for c in range(NC):
        m = core_of == c
        s_c, d_c = src[m], dst[m]
        eidx_c = np.nonzero(m)[0]
        rel = d_c - c * NPC
        w_c = rel // WIN
        order = np.argsort(w_c, kind="stable")
        s_c, rel, w_c, eidx_c = s_c[order], rel[order], w_c[order], eidx_c[order]
        bounds = np.searchsorted(w_c, np.arange(NW + 1))
        for w in range(NW):
            a, b = bounds[w], bounds[w + 1]
            per[c][w] = (s_c[a:b], rel[a:b] - w * WIN, eidx_c[a:b])

    nt = np.zeros(NW, np.int64)
    for w in range(NW):
        nt[w] = max(max(-(-len(per[c][w][0]) // 128) for c in range(NC)), 1)

    # chunk windows so each chunk is <= CHUNK_TILES tiles
    chunks = []  # (first_window, n_windows, tile_offset, n_tiles)
    w0, t0, toff = 0, 0, 0
    for w in range(NW):
        t = int(nt[w])
        if t0 and t0 + t > CHUNK_TILES:
            chunks.append((w0, w - w0, toff, t0))
            toff += t0
            w0, t0 = w, t
        else:
            t0 += t
    chunks.append((w0, NW - w0, toff, t0))

    starts = np.concatenate([[0], np.cumsum(nt)])
    return dict(per=per, nt=nt, starts=starts, chunks=chunks,
                total_tiles=int(nt.sum()))


def _per_core_arrays(plan, c, x_bf, ea_bf):
    nt, starts = plan["nt"], plan["starts"]
    T = plan["total_tiles"]
    xsrcT = np.zeros((128, T * 128), BF)
    eattrT = np.zeros((128, T * 128), BF)
    ohP = np.zeros((128, T * 128), BF)
    ohT = np.zeros((128, T * 128), BF)

    for w in range(NW):
        s, rel, eix = plan["per"][c][w]
        k = len(s)
        col0 = int(starts[w]) * 128
        xsrcT[:, col0:col0 + k] = x_bf[s].T
        eattrT[:, col0:col0 + k] = ea_bf[eix].T
        j = np.arange(k)
        ohP[j % 128, (int(starts[w]) + j // 128) * 128 + rel] = 1.0
        ohT[rel, col0 + j] = 1.0

    return dict(xsrcT=xsrcT, eattrT=eattrT, ohP=ohP, ohT=ohT)


def _build(plan):
    nt = plan["nt"]
    T = plan["total_tiles"]

    nc = bacc.Bacc("TRN2", target_bir_lowering=False, debug=False,
                   num_devices=NC)

    def din(name, shape, dt):
        return nc.declare_dram_parameter(name, list(shape), dt, isOutput=False)

    xTn = din("xTn", (128, NPAD), BF16)
    xsrcT = din("xsrcT", (128, T * 128), BF16)
    eattrT = din("eattrT", (128, T * 128), BF16)
    ohP_d = din("ohP", (128, T * 128), BF16)
    ohT_d = din("ohT", (128, T * 128), BF16)
    wkvT = din("wkvT", (128, 256), BF16)
    we2T = din("we2T", (128, 256), BF16)
    wqT = din("wqT", (128, 128), BF16)
    woT = din("woT", (128, 128), BF16)
    wpT = din("wpT", (128, 128), BF16)
    proto = din("proto", (128, 1), BF16)
    bo_c = din("bo", (128, 1), F32)
    gnw_c = din("gnw", (128, 1), F32)
    gnb_c = din("gnb", (128, 1), F32)
    gms_c = din("gms", (128, 1), F32)
    ones_row = din("ones_row", (1, 128), BF16)
    ident = din("ident", (128, 128), BF16)

    out_d = nc.declare_dram_parameter("out", [128, NPC], F32, isOutput=True)

    st_in = nc.dram_tensor("st_in", [128, 2], F32)
    st_out = nc.dram_tensor("st_out", [128, 2], F32, addr_space="Shared")

    with tile.TileContext(nc) as tc:
        with (
            tc.tile_pool(name="const", bufs=1) as cst,
            tc.tile_pool(name="persist", bufs=1) as psst,
        ):
            def const(name, dram, shape, dt):
                t = cst.tile(list(shape), dt, tag=name)
                nc.sync.dma_start(out=t[:], in_=dram[:])
                return t

            wkvT_s = const("wkvT", wkvT, (128, 256), BF16)
            we2T_s = const("we2T", we2T, (128, 256), BF16)
            wqT_s = const("wqT", wqT, (128, 128), BF16)
            woT_s = const("woT", woT, (128, 128), BF16)
            wpT_s = const("wpT", wpT, (128, 128), BF16)
            proto_s = const("proto", proto, (128, 1), BF16)
            ones_s = const("ones", ones_row, (1, 128), BF16)
            ident_s = const("ident", ident, (128, 128), BF16)
            bo_s = const("bo", bo_c, (128, 1), F32)
            gnw_s = const("gnw", gnw_c, (128, 1), F32)
            gnb_s = const("gnb", gnb_c, (128, 1), F32)
            gms_s = const("gms", gms_c, (128, 1), F32)

            outT = psst.tile([128, NPAD], F32, tag="outT")
            q_sb = psst.tile([128, NW, 128], BF16, tag="qsb")
            xr_sb = psst.tile([128, NPAD], BF16, tag="xrsb")
            nc.sync.dma_start(out=xr_sb[:], in_=xTn[:])

            # ---- prototype projection (once) ------------------------
            with tc.tile_pool(name="qinit", bufs=1, space="PSUM") as qip:
                p_ps = qip.tile([1, 128], F32, space="PSUM", tag="pps")
                nc.tensor.matmul(out=p_ps[:], lhsT=proto_s[:], rhs=wpT_s[:],
                                 start=True, stop=True)
                p_row = cst.tile([1, 128], BF16, tag="prow")
                nc.vector.tensor_copy(out=p_row[:], in_=p_ps[:])
                pbc_ps = qip.tile([128, 128], F32, space="PSUM", tag="pbcps")
                nc.tensor.matmul(out=pbc_ps[:], lhsT=ones_s[:], rhs=p_row[:],
                                 start=True, stop=True)
                pbc = cst.tile([128, 128], BF16, tag="pbc")
                nc.vector.tensor_copy(out=pbc[:], in_=pbc_ps[:])

            # ---- edge phase -----------------------------------------
            with (
                tc.tile_pool(name="xs", bufs=2) as xsp,
                tc.tile_pool(name="ea", bufs=2) as eap,
                tc.tile_pool(name="oh", bufs=2) as ohp,
                tc.tile_pool(name="ohT", bufs=2) as ohTp,
                tc.tile_pool(name="wk", bufs=4) as wk,
                tc.tile_pool(name="wcl", bufs=2) as wcl,
                tc.tile_pool(name="kvps", bufs=2, space="PSUM") as kvps,
                tc.tile_pool(name="qeps", bufs=2, space="PSUM") as qeps,
                tc.tile_pool(name="aggp", bufs=2, space="PSUM") as aggp,
            ):
                pending = []  # (macro_id, kind, args) deferred by SKEW macros
                SKEW = 2

                def flush(upto):
                    while pending and pending[0][0] <= upto:
                        _, kind, args = pending.pop(0)
                        if kind == "agg":
                            aggbank_, oh_, ti_, wmsg_, t_, first_, last_ = \
                                args
                            nc.tensor.matmul(
                                out=aggbank_[:, 0:136], lhsT=oh_[:, ti_, :],
                                rhs=wmsg_[:, t_, :], start=first_,
                                stop=last_)
                        else:
                            close_window(*args)

                def emit_q(w):
                    q0 = qeps.tile([128, 128], F32, space="PSUM", tag="q0")
                    nc.tensor.matmul(out=q0[:],
                                     lhsT=xr_sb[:, w * 128:(w + 1) * 128],
                                     rhs=wqT_s[:], start=True, stop=True)
                    pp = wcl.tile([128, 128], F32, tag="pp")
                    nc.vector.tensor_tensor(out=pp[:], in0=q0[:], in1=pbc[:],
                                            op=OP.mult)
                    al = wcl.tile([128, 8], F32, tag="al")
                    nc.vector.tensor_reduce(
                        out=al[:],
                        in_=pp[:].rearrange("p (h d) -> p h d", h=8),
                        axis=mybir.AxisListType.X, op=OP.add)
                    als = wcl.tile([128, 8], F32, tag="als")
                    nc.scalar.activation(out=als[:], in_=al[:], func=AF.Relu,
                                         scale=0.1)
                    t2 = wcl.tile([128, 128], F32, tag="t2")
                    nc.vector.tensor_tensor(
                        out=t2[:].rearrange("p (h d) -> p h d", h=8),
                        in0=pbc[:].rearrange("p (h d) -> p h d", h=8),
                        in1=als[:].rearrange("p (h o) -> p h o", o=1)
                        .to_broadcast([128, 8, 16]),
                        op=OP.mult)
                    q3 = wcl.tile([128, 128], F32, tag="q3")
                    nc.vector.tensor_tensor(out=q3[:], in0=q0[:], in1=t2[:],
                                            op=OP.add)
                    nc.vector.tensor_scalar(out=q_sb[:, w, :], in0=q3[:],
                                            scalar1=0.25, scalar2=None,
                                            op0=OP.mult)

                def close_window(aggbank, w):
                    rs_e = wcl.tile([128, 8], F32, tag="rse")
                    nc.vector.tensor_scalar(out=rs_e[:],
                                            in0=aggbank[:, 128:136],
                                            scalar1=1e-6, scalar2=None,
                                            op0=OP.add)
                    rinv = wcl.tile([128, 8], F32, tag="rinv")
                    nc.vector.reciprocal(out=rinv[:], in_=rs_e[:])
                    aggn = wcl.tile([128, 128], BF16, tag="aggn")
                    nc.vector.tensor_tensor(
                        out=aggn[:].rearrange("p (h d) -> p h d", h=8),
                        in0=aggbank[:, 0:128].rearrange("p (h d) -> p h d",
                                                        h=8),
                        in1=rinv[:].rearrange("p (h o) -> p h o", o=1)
                        .to_broadcast([128, 8, 16]),
                        op=OP.mult)
                    at_ps = aggbank[:, 192:256].bitcast(BF16)
                    nc.tensor.transpose(out=at_ps, in_=aggn[:],
                                        identity=ident_s[:])
                    at_sb = wcl.tile([128, 128], BF16, tag="atsb")
                    nc.scalar.activation(out=at_sb[:], in_=at_ps,
                                         func=AF.Copy)
                    ow_ps = aggbank[:, 256:384]
                    nc.tensor.matmul(out=ow_ps, lhsT=woT_s[:],
                                     rhs=at_sb[:], start=True, stop=True)
                    ow1 = wcl.tile([128, 128], F32, tag="ow1")
                    nc.scalar.activation(out=ow1[:], in_=ow_ps[:],
                                         func=AF.Identity, bias=bo_s[:, 0:1])
                    nc.vector.tensor_tensor(
                        out=outT[:, w * 128:(w + 1) * 128],
                        in0=ow1[:],
                        in1=xr_sb[:, w * 128:(w + 1) * 128], op=OP.add)

                gm = 0  # global macro counter
                first_chunk = True
                for (w0, nwin, toff, ctiles) in plan["chunks"]:
                    c0 = toff * 128
                    cn = ctiles * 128
                    xs = xsp.tile([128, CHUNK_TILES * 128], BF16, tag="xs")
                    nc.sync.dma_start(out=xs[:, :cn],
                                      in_=xsrcT[:, c0:c0 + cn])
                    ea = eap.tile([128, CHUNK_TILES * 128], BF16, tag="ea")
                    nc.sync.dma_start(out=ea[:, :cn],
                                      in_=eattrT[:, c0:c0 + cn])
                    oh = ohp.tile([128, CHUNK_TILES, 128], BF16, tag="oh")
                    nc.sync.dma_start(
                        out=oh[:, :ctiles, :].rearrange("p t n -> p (t n)"),
                        in_=ohP_d[:, c0:c0 + cn])
                    ohT = ohTp.tile([128, CHUNK_TILES, 128], BF16, tag="ohT")
                    nc.sync.dma_start(
                        out=ohT[:, :ctiles, :].rearrange("p t n -> p (t n)"),
                        in_=ohT_d[:, c0:c0 + cn])
                    if first_chunk:
                        emit_q(0)
                        first_chunk = False

                    ct = 0  # tile index within chunk
                    for w in range(w0, w0 + nwin):
                        wt = int(nt[w])
                        aggbank = aggp.tile([128, 512], F32, space="PSUM")
                        wtile = 0
                        while wtile < wt:
                            mt = min(MACRO, wt - wtile)
                            kv_ps = kvps.tile([128, MACRO, 256], F32,
                                              space="PSUM")
                            for t in range(mt):
                                col = (ct + t) * 128
                                nc.tensor.matmul(
                                    out=kv_ps[:, t, :],
                                    lhsT=xs[:, col:col + 128],
                                    rhs=wkvT_s[:], start=True, stop=False)
                                nc.tensor.matmul(
                                    out=kv_ps[:, t, :],
                                    lhsT=ea[:, col:col + 128],
                                    rhs=we2T_s[:], start=False, stop=True)
                            qe_ps = qeps.tile([128, MACRO * 128], F32,
                                              space="PSUM", tag="qe")
                            for t in range(mt):
                                nc.tensor.matmul(
                                    out=qe_ps[:, t * 128:(t + 1) * 128],
                                    lhsT=ohT[:, ct + t, :],
                                    rhs=q_sb[:, w, :], start=True, stop=True)
                            if wtile == 0 and w + 1 < NW:
                                emit_q(w + 1)
                            flush(gm - SKEW)

                            kvpe_sb = wk.tile([128, MACRO, 256], BF16,
                                              tag="kvpesb")
                            nc.scalar.activation(out=kvpe_sb[:, :mt, :],
                                                 in_=kv_ps[:, :mt, :],
                                                 func=AF.Copy)
                            prod = wk.tile([128, MACRO, 128], BF16,
                                           tag="prod")
                            nc.vector.tensor_tensor(
                                out=prod[:, :mt, :],
                                in0=qe_ps[:, :mt * 128]
                                .rearrange("p (t r) -> p t r", t=mt),
                                in1=kvpe_sb[:, :mt, 0:128],
                                op=OP.mult)
                            score = wk.tile([128, MACRO * 8], F32,
                                            tag="score")
                            nc.vector.tensor_reduce(
                                out=score[:, :mt * 8],
                                in_=prod[:, :mt, :]
                                .rearrange("p t (h d) -> p (t h) d", h=8),
                                axis=mybir.AxisListType.X, op=OP.add)
                            wmsg = wk.tile([128, MACRO, 136], BF16,
                                           tag="wmsg")
                            nc.scalar.activation(
                                out=wmsg[:, :mt, 128:136],
                                in_=score[:, :mt * 8]
                                .rearrange("p (t h) -> p t h", h=8),
                                func=AF.Relu)
                            nc.gpsimd.tensor_tensor(
                                out=wmsg[:, :mt, 0:128]
                                .rearrange("p t (h d) -> p t h d", h=8),
                                in0=kvpe_sb[:, :mt, 128:256]
                                .rearrange("p t (h d) -> p t h d", h=8),
                                in1=wmsg[:, :mt, 128:136]
                                .rearrange("p t (h o) -> p t h o", h=8, o=1)
                                .to_broadcast([128, mt, 8, 16]),
                                op=OP.mult)
                            for t in range(mt):
                                last = (wtile + t == wt - 1)
                                pending.append(
                                    (gm, "agg",
                                     (aggbank, oh, ct + t, wmsg, t,
                                      wtile + t == 0, last)))
                            wtile += mt
                            ct += mt
                            gm += 1
                        pending.append((gm - 1, "close", (aggbank, w)))
                flush(gm)

                # ---- GraphNorm -------------------------------------
                s1 = wcl.tile([128, 1], F32, tag="s1")
                nc.vector.tensor_reduce(out=s1[:], in_=outT[:, :NPC],
                                        axis=mybir.AxisListType.X, op=OP.add)
                s2 = wcl.tile([128, 1], F32, tag="s2")
                s2p = wcl.tile([128, 1], F32, tag="s2p")
                nc.vector.memset(s2[:], 0.0)
                for ch in range(0, NPC, 512):
                    cw = min(512, NPC - ch)
                    sq = wk.tile([128, 512], F32, tag="sq")
                    nc.scalar.activation(out=sq[:, :cw],
                                         in_=outT[:, ch:ch + cw],
                                         func=AF.Square)
                    nc.vector.tensor_reduce(out=s2p[:], in_=sq[:, :cw],
                                            axis=mybir.AxisListType.X,
                                            op=OP.add)
                    nc.vector.tensor_tensor(out=s2[:], in0=s2[:], in1=s2p[:],
                                            op=OP.add)
                st_sb = wcl.tile([128, 2], F32, tag="stsb")
                nc.vector.tensor_copy(out=st_sb[:, 0:1], in_=s1[:])
                nc.vector.tensor_copy(out=st_sb[:, 1:2], in_=s2[:])
                nc.sync.dma_start(out=st_in[:], in_=st_sb[:])
                nc.gpsimd.collective_compute(
                    "AllReduce", OP.add, replica_groups=[list(range(NC))],
                    ins=[st_in[:]], outs=[st_out[:]])
                stg = wcl.tile([128, 2], F32, tag="stg")
                nc.sync.dma_start(out=stg[:], in_=st_out[:])

                mean = wcl.tile([128, 1], F32, tag="mean")
                nc.vector.tensor_scalar(out=mean[:], in0=stg[:, 0:1],
                                        scalar1=1.0 / N, scalar2=None,
                                        op0=OP.mult)
                m2 = wcl.tile([128, 1], F32, tag="m2")
                nc.vector.tensor_scalar(out=m2[:], in0=stg[:, 1:2],
                                        scalar1=1.0 / N, scalar2=None,
                                        op0=OP.mult)
                gm = wcl.tile([128, 1], F32, tag="gm")
                nc.vector.tensor_tensor(out=gm[:], in0=gms_s[:], in1=mean[:],
                                        op=OP.mult)
                var = wcl.tile([128, 1], F32, tag="var")
                nc.vector.tensor_tensor(out=var[:], in0=gm[:], in1=gm[:],
                                        op=OP.mult)
                tmp = wcl.tile([128, 1], F32, tag="tmp")
                nc.vector.tensor_tensor(out=tmp[:], in0=gm[:], in1=mean[:],
                                        op=OP.mult)
                nc.vector.tensor_scalar(out=tmp[:], in0=tmp[:], scalar1=-2.0,
                                        scalar2=None, op0=OP.mult)
                nc.vector.tensor_tensor(out=var[:], in0=var[:], in1=tmp[:],
                                        op=OP.add)
                nc.vector.tensor_tensor(out=var[:], in0=var[:], in1=m2[:],
                                        op=OP.add)
                nc.vector.tensor_scalar(out=var[:], in0=var[:], scalar1=1e-5,
                                        scalar2=None, op0=OP.add)
                std = wcl.tile([128, 1], F32, tag="std")
                nc.scalar.sqrt(out=std[:], in_=var[:])
                rstd = wcl.tile([128, 1], F32, tag="rstd")
                nc.vector.reciprocal(out=rstd[:], in_=std[:])
                acol = wcl.tile([128, 1], F32, tag="acol")
                nc.vector.tensor_tensor(out=acol[:], in0=gnw_s[:],
                                        in1=rstd[:], op=OP.mult)
                bcol = wcl.tile([128, 1], F32, tag="bcol")
                nc.vector.tensor_tensor(out=bcol[:], in0=acol[:], in1=gm[:],
                                        op=OP.mult)
                nc.vector.tensor_scalar(out=bcol[:], in0=bcol[:],
                                        scalar1=-1.0, scalar2=None,
                                        op0=OP.mult)
                nc.vector.tensor_tensor(out=bcol[:], in0=bcol[:],
                                        in1=gnb_s[:], op=OP.add)

                fin = psst.tile([128, NPC], F32, tag="fin")
                nc.vector.tensor_scalar(out=fin[:], in0=outT[:, :NPC],
                                        scalar1=acol[:, 0:1],
                                        scalar2=bcol[:, 0:1],
                                        op0=OP.mult, op1=OP.add)
                nc.vector.tensor_scalar(out=fin[:], in0=fin[:], scalar1=0.0,
                                        scalar2=None, op0=OP.max)
                nc.sync.dma_start(out=out_d[:], in_=fin[:])

    nc.compile()
    return nc


def _in_maps(plan, x, edge_attr, prototype, WQ, WK, WV, WE, Wp, Wo, bo,
             gn_weight, gn_bias, gn_mean_scale):
    x_bf = np.asarray(x, np.float32).astype(BF)
    ea_bf = np.asarray(edge_attr, np.float32).astype(BF)
    wkvT = np.concatenate([np.asarray(WK, np.float32).T,
                           np.asarray(WV, np.float32).T], axis=1).astype(BF)
    weT = np.asarray(WE, np.float32).T.astype(BF)
    we2T = np.concatenate([weT, weT], axis=1)
    consts = dict(
        wkvT=np.ascontiguousarray(wkvT),
        we2T=np.ascontiguousarray(we2T),
        wqT=np.ascontiguousarray(np.asarray(WQ, np.float32).T).astype(BF),
        woT=np.ascontiguousarray(np.asarray(Wo, np.float32).T).astype(BF),
        wpT=np.ascontiguousarray(np.asarray(Wp, np.float32).T).astype(BF),
        proto=np.asarray(prototype, np.float32).reshape(128, 1).astype(BF),
        bo=np.asarray(bo, np.float32).reshape(128, 1),
        gnw=np.asarray(gn_weight, np.float32).reshape(128, 1),
        gnb=np.asarray(gn_bias, np.float32).reshape(128, 1),
        gms=np.asarray(gn_mean_scale, np.float32).reshape(128, 1),
        ones_row=np.ones((1, 128), BF),
        ident=np.eye(128, dtype=BF),
    )
    maps = []
    for c in range(NC):
        arrs = _per_core_arrays(plan, c, x_bf, ea_bf)
        pad = np.zeros((NPAD, 128), BF)
        pad[:NPC] = x_bf[c * NPC:(c + 1) * NPC]
        m = dict(consts)
        m["xTn"] = np.ascontiguousarray(pad.T)
        m["xsrcT"] = arrs["xsrcT"]
        m["eattrT"] = arrs["eattrT"]
        m["ohP"] = arrs["ohP"]
        m["ohT"] = arrs["ohT"]
        maps.append(m)
    return maps


def kernel(x, edge_attr, prototype, WQ, WK, WV, WE, Wp, Wo, bo,
           gn_weight, gn_bias, gn_mean_scale, edge_index):
    x = np.asarray(x, np.float32)
    edge_attr = np.asarray(edge_attr, np.float32)
    plan = _plan(np.asarray(edge_index))
    nc = _build(plan)
    maps = _in_maps(plan, x, edge_attr, prototype, WQ, WK, WV, WE, Wp, Wo,
                    bo, gn_weight, gn_bias, gn_mean_scale)
    res = run_bass_kernel_spmd(nc, maps, list(range(NC)), trace=False)
    out = np.empty((N, DIM), np.float32)
    for c in range(NC):
        out[c * NPC:(c + 1) * NPC] = res.results[c]["out"].T
    return out
